# revision 2
# baseline (speedup 1.0000x reference)
"""Minkowski sparse conv-transpose, v2 — merged indirect DMAs + rank-slice
bypass scatter + on-device reduction.

Sharding: output rows are range-sharded (50000/core). Per core:
  - pairs grouped by kernel offset k, chunked into bursts of Q=2048;
  - one merged indirect gather per burst (2048 descriptors, one instruction);
  - per 16-slot group: DVE 32x32 block transpose, 4x 32x32 TensorE matmuls
    (tile_position quadrants), transpose back;
  - one merged indirect scatter per burst writes values (BYPASS, no CCE) into
    a rank-sliced scratch buffer: pair (row, occurrence idx rho) -> slot
    off[rho] + pos[row], where rows are host-permuted by multiplicity
    descending so slice rho is a prefix of length c_rho. All (row,rho) slots
    are globally unique => every scatter is conflict-free and fully
    concurrent (fake dep offsets).
  - pad pairs gather appended zero rows of feats and scatter into the unused
    slice-tail slots (zeroing them) or trash; scratch thus needs no zero-init.
  - after a barrier, a tiled reduction sums the rank slices (sequential DMA +
    DVE adds) and writes the output rows once, sequentially.
Host assembles the 8 core slices and undoes the per-core row permutation.
"""
import numpy as np

import concourse.bass as bass
import concourse.mybir as mybir
import concourse.tile as tile
from concourse.bass_utils import run_bass_kernel_spmd

dt = mybir.dt

NCORES = 8
K = 27
N_IN = 200000
N_OUT = 400000
C = 32
RPC = N_OUT // NCORES  # 50000
Q = 2048               # pairs per burst = descriptors per indirect DMA
SLOT = Q // 128        # 16 index columns per burst half
TR = 2048              # reduction tile rows; rank slices padded to TR
TRASH = 32768
NZ = 2048              # zero rows appended to feats


def _split_dma_waits(nc, max_waits=1):
    """This toolchain allows only one sync wait per instruction; hoist
    extras onto a chain of single-wait NoOps ahead of the instruction."""
    for bb in nc.main_func.blocks:
        out = []
        for ins in bb.instructions:
            if ins.sync_info is not None and len(ins.sync_info.on_wait) > max_waits:
                waits = list(ins.sync_info.on_wait)
                extra, keep = waits[:-max_waits], waits[-max_waits:]
                for i, w in enumerate(extra):
                    nop = mybir.InstNoOp(name=f"{ins.name}-ws{i}", ins=[], outs=[])
                    nop.engine = ins.engine
                    nop.sync_info = mybir.SyncInfo(on_wait=[w], on_update=[])
                    out.append(nop)
                ins.sync_info = mybir.SyncInfo(
                    on_wait=keep, on_update=list(ins.sync_info.on_update)
                )
            out.append(ins)
        bb.instructions[:] = out


def _prep(in_map, out_map):
    """Host-side index preprocessing. Returns (meta, per-core widx, orders)."""
    cores = []
    for c in range(NCORES):
        lo = c * RPC
        per_k = []
        for k in range(K):
            om = out_map[k]
            sel = (om >= lo) & (om < lo + RPC)
            gi = in_map[k][sel].astype(np.int64)
            lr = (om[sel] - lo).astype(np.int64)
            per_k.append((gi, lr))
        cores.append(per_k)

    info = []
    for c in range(NCORES):
        all_lr = np.concatenate([lr for _, lr in cores[c]])
        mult = np.bincount(all_lr, minlength=RPC)
        order = np.argsort(-mult, kind="stable")
        pos = np.empty(RPC, np.int64)
        pos[order] = np.arange(RPC)
        s = np.argsort(all_lr, kind="stable")
        lr_s = all_lr[s]
        new = np.ones(lr_s.size, bool)
        new[1:] = lr_s[1:] != lr_s[:-1]
        seg_start = np.where(new)[0]
        seg_id = np.cumsum(new) - 1
        rank_s = np.arange(lr_s.size) - seg_start[seg_id]
        ranks = np.empty(all_lr.size, np.int64)
        ranks[s] = rank_s
        info.append((mult, order, pos, ranks))

    RMAX = int(max(i[0].max() for i in info))
    c_u = np.zeros(RMAX, np.int64)
    c_core = np.zeros((NCORES, RMAX), np.int64)
    for c in range(NCORES):
        mult = info[c][0]
        cnt = np.bincount(np.minimum(mult, RMAX), minlength=RMAX + 1)
        csum = np.cumsum(cnt)
        for rho in range(RMAX):
            c_core[c, rho] = RPC - csum[rho]
    c_u = c_core.max(axis=0)
    cpad = ((c_u + TR - 1) // TR) * TR
    off = np.zeros(RMAX + 1, np.int64)
    off[1:] = np.cumsum(cpad)
    bufrows = int(off[RMAX] + TRASH)

    n_k_max = [max(len(cores[c][k][0]) for c in range(NCORES)) for k in range(K)]
    nb_k = [max(1, (n + Q - 1) // Q) for n in n_k_max]
    # ensure enough pad descriptors to zero-fill every slice tail on all cores
    fills_max = int(max((cpad[None, :] - c_core[c]).sum() for c in range(NCORES)))
    while True:
        pads_min = min(
            sum(nb_k[k] * Q - len(cores[c][k][0]) for k in range(K))
            for c in range(NCORES)
        )
        if pads_min >= fills_max:
            break
        nb_k[K - 1] += 1
    nb_total = sum(nb_k)

    widxs = []
    for c in range(NCORES):
        mult, order, pos, ranks = info[c]
        fill = np.concatenate(
            [off[rho] + np.arange(c_core[c, rho], cpad[rho]) for rho in range(RMAX)]
        ) if RMAX else np.empty(0, np.int64)
        fi = 0
        gz = [0]
        tz = [0]
        widx = np.empty((nb_total, 128, 2 * SLOT), np.int32)
        b0 = 0
        pair_ofs = 0
        for k in range(K):
            gi, lr = cores[c][k]
            n = gi.size
            rk = ranks[pair_ofs:pair_ofs + n]
            slots = off[rk] + pos[lr]
            pair_ofs += n
            tgt = nb_k[k] * Q
            npad = tgt - n
            gpad = N_IN + ((gz[0] + np.arange(npad)) % NZ)
            gz[0] += npad
            take = min(npad, fill.size - fi)
            ntr = npad - take
            spad = np.concatenate([
                fill[fi:fi + take],
                off[RMAX] + ((tz[0] + np.arange(ntr)) % TRASH),
            ])
            tz[0] += ntr
            fi += take
            g_all = np.concatenate([gi, gpad])
            s_all = np.concatenate([slots, spad])
            for b in range(nb_k[k]):
                widx[b0 + b, :, :SLOT] = g_all[b * Q:(b + 1) * Q].reshape(128, SLOT)
                widx[b0 + b, :, SLOT:] = s_all[b * Q:(b + 1) * Q].reshape(128, SLOT)
            b0 += nb_k[k]
        assert fi == fill.size, (fi, fill.size)
        widxs.append(widx)

    orders = [info[c][1] for c in range(NCORES)]
    meta = (tuple(nb_k), tuple(int(x) for x in cpad), bufrows)
    return meta, widxs, orders


_CACHE = {}
_LAST_IN_MAPS = None


def _build(meta):
    if meta in _CACHE:
        return _CACHE[meta]
    nb_k, cpad, bufrows = meta
    nb_total = sum(nb_k)
    RMAX = len(cpad)
    off = np.zeros(RMAX + 1, np.int64)
    off[1:] = np.cumsum(cpad)
    ACC2 = int(cpad[0])

    nc = bass.Bass(num_swdge_queues=2)
    featsz = nc.declare_dram_parameter("featsz", [N_IN + NZ, C], dt.float32, isOutput=False)
    wstack = nc.declare_dram_parameter("wstack", [K, 128, C], dt.float32, isOutput=False)
    widx = nc.declare_dram_parameter("widx", [nb_total, 128, 2 * SLOT], dt.int32, isOutput=False)
    acc = nc.declare_dram_parameter("acc", [ACC2, C], dt.float32, isOutput=True)
    buf = nc.dram_tensor("scratch", (bufrows, C), dt.float32, kind="Internal")

    with tile.TileContext(nc) as tc:
        with (
            tc.tile_pool(name="sbuf", bufs=8) as sb,
            tc.tile_pool(name="wpool", bufs=2) as wp,
            tc.tile_pool(name="rpool", bufs=4) as rp,
            tc.tile_pool(name="psum", bufs=4, space="PSUM") as ps,
        ):
            fake = [0]
            b = 0
            for k in range(K):
                w4 = wp.tile([128, C], dt.float32, tag="w4")
                nc.sync.dma_start(out=w4[:], in_=wstack[k])
                for _ in range(nb_k[k]):
                    wi = sb.tile([128, 2 * SLOT], dt.int32, tag="wi")
                    nc.sync.dma_start(out=wi[:], in_=widx[b])
                    x = sb.tile([128, SLOT, C], dt.float32, tag="x")
                    for j in range(SLOT):
                        nc.gpsimd.indirect_dma_start(
                            out=x[:, j, :],
                            out_offset=None,
                            in_=featsz[0:128, :],
                            in_offset=bass.IndirectOffsetOnAxis(
                                ap=wi[:, j:j + 1], axis=0
                            ),
                        )
                    xt = sb.tile([128, SLOT * C], dt.float32, tag="xt")
                    nc.vector.transpose(out=xt[:], in_=x[:])
                    pt = ps.tile([128, SLOT * C], dt.float32, tag="pt")
                    for r in range(4):
                        nc.tensor.matmul(
                            out=pt[32 * r:32 * r + 32, :],
                            lhsT=w4[32 * r:32 * r + 32, :],
                            rhs=xt[32 * r:32 * r + 32, :],
                            start=True,
                            stop=True,
                            tile_position=(32 * r, 32 * r),
                        )
                    v = sb.tile([128, SLOT, C], dt.float32, tag="v")
                    nc.vector.transpose(out=v[:], in_=pt[:])
                    for j in range(SLOT):
                        fake[0] += 1
                        oap = buf[0:128, :]
                        oap = bass.AP(
                            tensor=oap.tensor, offset=oap.offset, ap=oap.ap,
                            dep_tracking_offset=fake[0] * bufrows * C,
                        )
                        s_inst = nc.gpsimd.indirect_dma_start(
                            out=oap,
                            out_offset=bass.IndirectOffsetOnAxis(
                                ap=wi[:, SLOT + j:SLOT + j + 1], axis=0
                            ),
                            in_=v[:, j, :],
                            in_offset=None,
                        )
                        s_inst.ins.queue = "qPoolDynamic1"
                    b += 1
            tc.strict_bb_all_engine_barrier()
            CW = TR * C // 128  # 512
            for t in range(ACC2 // TR):
                r0 = t * TR
                at = rp.tile([128, CW], dt.float32, tag="at")
                nc.sync.dma_start(out=at[:], in_=buf[int(off[0]) + r0:int(off[0]) + r0 + TR, :])
                for rho in range(1, RMAX):
                    if cpad[rho] <= r0:
                        break
                    st = rp.tile([128, CW], dt.float32, tag="st")
                    nc.sync.dma_start(out=st[:], in_=buf[int(off[rho]) + r0:int(off[rho]) + r0 + TR, :])
                    nc.vector.tensor_tensor(
                        out=at[:], in0=at[:], in1=st[:], op=mybir.AluOpType.add
                    )
                nc.sync.dma_start(out=acc[r0:r0 + TR, :], in_=at[:])
    _split_dma_waits(nc)
    _CACHE[meta] = nc
    return nc


def kernel(feats, kernel, in_map, out_map, n_out):
    feats = np.asarray(feats, dtype=np.float32)
    wk = np.asarray(kernel, dtype=np.float32)
    in_map = np.asarray(in_map, dtype=np.int32)
    out_map = np.asarray(out_map, dtype=np.int32)

    meta, widxs, orders = _prep(in_map, out_map)
    nc = _build(meta)

    featsz = np.ascontiguousarray(
        np.concatenate([feats, np.zeros((NZ, C), np.float32)], axis=0)
    )
    wstack = np.tile(wk, (1, 4, 1)).reshape(K, 128, C).astype(np.float32)
    in_maps = [
        dict(featsz=featsz, wstack=wstack, widx=np.ascontiguousarray(widxs[c]))
        for c in range(NCORES)
    ]

    global _LAST_IN_MAPS
    _LAST_IN_MAPS = in_maps
    res = run_bass_kernel_spmd(nc, in_maps, list(range(NCORES)))

    out = np.empty((N_OUT, C), np.float32)
    for c in range(NCORES):
        a = res.results[c]["acc"]
        oc = np.empty((RPC, C), np.float32)
        oc[orders[c]] = a[:RPC]
        out[c * RPC:(c + 1) * RPC] = oc
    return out
